# revision 44
# baseline (speedup 1.0000x reference)
"""Trainium2 Bass kernel for nn_EnhancedQuantumLayer (10-qubit, 4-layer
variational circuit, batch 512, Z-expectations output).

Strategy (data parallel over 8 cores, 64 samples/core):
  - Feature map is a product state: per (sample, qubit) 2-vector v computed
    by a 10-step RZ/RX recursion on [64, 40] tiles (DVE/Pool) with sin/cos
    planes from the ACT engine.
  - Statevector [64, 1024] (complex -> separate re/im fp32 planes) held as
    [128, 512] tiles: layout A: partition = (b5, q4, b4, h4), free =
    (u4, l5); layout B (after 32x32 StreamTranspose): partition =
    (b5, q4, l5), free = (u4, b4, h4).  h = q0..q3 (+q4 in partitions),
    l = q5..q9, sample b = (b5, b4, u4).
  - Repack from sample-major g [64, 64] to layout A via one SBUF->SBUF DMA
    (lpp) + E4 replication matmul for the L half, and StreamTranspose +
    engine copies for the H half (hp).  Layer-0 A-side gates are applied
    to the tiny hp [128, 16] before the H*L outer product (valid since
    S_A acts only on (q4, h4) and L is constant there).
  - Each layer: 4 fp32r matmuls per side, DVE StreamTranspose between
    layouts; CZCNOT entanglers folded into stationaries on host; final
    A-entanglers folded into the measurement signs.
  - Measurement: |amp|^2, W1 sign matmul over partitions, sign-weighted
    free-dim reductions on DVE, one 32x32 transpose, 2 output DMAs.

Host precompute is theta-only (24 128x128 stationaries) -- O(1) in batch.
"""

import numpy as np

N_QUBITS = 10
N_LAYERS = 4
FREQS = (1.0, 2.0, 4.0, 8.0, 16.0)
PI = float(np.pi)
B_TOTAL = 512
B_CORE = 64
N_CORES = 8

H_QUBITS = [4, 0, 1, 2, 3]   # kron order (MSB first) for h index
L_QUBITS = [5, 6, 7, 8, 9]

CZCNOT = np.array([[1, 0, 0, 0],
                   [0, 1, 0, 0],
                   [0, 0, 0, -1],
                   [0, 0, 1, 0]], dtype=np.complex128)


# ---------------------------------------------------------------- host math
def _rz(phi):
    return np.array([[np.exp(-0.5j * phi), 0], [0, np.exp(0.5j * phi)]],
                    dtype=np.complex128)


def _rx(th):
    c, s = np.cos(th / 2), np.sin(th / 2)
    return np.array([[c, -1j * s], [-1j * s, c]], dtype=np.complex128)


def _ry(th):
    c, s = np.cos(th / 2), np.sin(th / 2)
    return np.array([[c, -s], [s, c]], dtype=np.complex128)


def _kron_list(ms):
    out = ms[0]
    for m in ms[1:]:
        out = np.kron(out, m)
    return out


def _embed_2q(space_qubits, qa, qb, M4):
    n = len(space_qubits)
    dim = 2 ** n
    pa, pb = space_qubits.index(qa), space_qubits.index(qb)
    out = np.zeros((dim, dim), dtype=np.complex128)
    for idx in range(dim):
        bits = [(idx >> (n - 1 - i)) & 1 for i in range(n)]
        col4 = 2 * bits[pa] + bits[pb]
        for row4 in range(4):
            val = M4[row4, col4]
            if val != 0:
                nb = bits.copy()
                nb[pa], nb[pb] = row4 >> 1, row4 & 1
                ridx = sum(bit << (n - 1 - i) for i, bit in enumerate(nb))
                out[ridx, idx] += val
    return out


A6 = [4, "b4", 0, 1, 2, 3]
L6 = [4, 5, 6, 7, 8, 9]
_E_evenA6 = _embed_2q(A6, 0, 1, CZCNOT) @ _embed_2q(A6, 2, 3, CZCNOT)
_E_oddA6 = _embed_2q(A6, 3, 4, CZCNOT) @ _embed_2q(A6, 1, 2, CZCNOT)
_PermA6 = _E_oddA6 @ _E_evenA6
_E_evenL6 = _embed_2q(L6, 6, 7, CZCNOT) @ _embed_2q(L6, 8, 9, CZCNOT)
_E_oddL6 = _embed_2q(L6, 7, 8, CZCNOT) @ _embed_2q(L6, 5, 6, CZCNOT)
_CG64 = _embed_2q(L6, 4, 5, CZCNOT)


def _layer_matrices6(theta):
    ang = np.tanh(theta.astype(np.float64)) * PI
    S_A, S_L = [], []
    for layer in range(N_LAYERS):
        U = []
        for q in range(10):
            a0, a1, a2 = ang[layer, q]
            U.append(_rx(a0 * 0.5) @ _rz(a2) @ _ry(a1) @ _rz(a0))
        UA6 = _kron_list([U[4], np.eye(2), U[0], U[1], U[2], U[3]])
        UL6 = _kron_list([np.eye(2), U[5], U[6], U[7], U[8], U[9]])
        S_A.append(UA6 if layer == 0 else UA6 @ _PermA6)
        S_L.append(_E_oddL6 @ _E_evenL6 @ _CG64 @ UL6)
    return S_A, S_L


def _host_weights(theta):
    """wstack [128, 24*128] fp32, pre-transposed so the device DMA is
    linear: wstack[p, 128*m + j] = mats[m][p, j], where mats[m] are per
    layer [Ar, Ain, Ai, Lr, Lin, Li], each kron(I2, S6).{comp}.T"""
    S_A, S_L = _layer_matrices6(theta)
    I2 = np.eye(2)
    mats = []
    for layer in range(N_LAYERS):
        for S in [S_A[layer], S_L[layer]]:
            full = np.kron(I2, S)
            mats.append(full.real.T)
            mats.append((-full.imag).T)
            mats.append(full.imag.T)
    stack = np.stack(mats).astype(np.float32)          # [24, 128, 128]
    return np.ascontiguousarray(
        stack.transpose(1, 0, 2).reshape(128, 24 * 128))


def _w1b():
    """W1b [128, 64]: stage-1 (layout B, partitions p = 64 b5 + 32 q4 + l).
    Col 32 b5 + j (j=0..4): sign of l bit j (qubits q5..q9); rest zero."""
    W1 = np.zeros((128, 64), dtype=np.float32)
    for b5 in range(2):
        for q4 in range(2):
            for l in range(32):
                p = 64 * b5 + 32 * q4 + l
                for j in range(5):
                    W1[p, 32 * b5 + j] = 1.0 - 2.0 * ((l >> (4 - j)) & 1)
    return W1


def _wA():
    """W_A [128, 64]: H-qubit measurement in layout A (partitions
    P = 64 b5 + 32 q4 + 16 b4 + h4).  Col 32 b5' + 16 b4' + q (q=0..4):
    [b5==b5'][b4==b4'] * chi_q(q4, h4) with the final A-entanglers folded:
    chi = (s0, s1 s0, s2 s1 s0, s3 s2, s3 s2 (1-2 q4))."""
    W = np.zeros((128, 64), dtype=np.float32)
    for b5 in range(2):
        for q4 in range(2):
            for h4 in range(16):
                s = [1.0 - 2.0 * ((h4 >> (3 - i)) & 1) for i in range(4)]
                chi = [s[0], s[1] * s[0], s[2] * s[1] * s[0], s[3] * s[2],
                       s[3] * s[2] * (1.0 - 2.0 * q4)]
                for b4 in range(2):
                    P = 64 * b5 + 32 * q4 + 16 * b4 + h4
                    for q in range(5):
                        W[P, 32 * b5 + 16 * b4 + q] = chi[q]
    return W


# ------------------------------------------------------------- bass builder
_BUILD_CACHE = {}

# cpack layout (one [128, 536] constant tensor, two DMAs):
#   part B (rows 0..127, cols 0..255): w1b [128,64] | wA [128,64] | e4 [4,128]
#   part A (rows 0..63, cols 256..536): v0 [64,40] | mult [64,120] |
#                                       bias [64,120]
_CA0 = 256          # start of feature-map consts (part A)


def _cpack():
    cp = np.zeros((128, 536), dtype=np.float32)
    # part B
    cp[:, 0:64] = _w1b()
    cp[:, 64:128] = _wA()
    e4 = np.zeros((4, 128), dtype=np.float32)
    for b5 in range(2):
        for b4 in range(2):
            for q4 in range(2):
                p0 = 64 * b5 + 32 * q4 + 16 * b4
                e4[2 * b5 + b4, p0:p0 + 16] = 1.0
    cp[0:4, 128:256] = e4
    # part A
    v0 = np.zeros((64, 40), dtype=np.float32)
    v0[:, 0::4] = 1.0  # alpha_re = 1
    cs_mult = np.zeros((12, 10), dtype=np.float32)
    cs_bias = np.zeros((12, 10), dtype=np.float32)
    for j in range(6):
        cs_mult[j] = 0.25 * 2 ** j
        cs_mult[6 + j] = 0.25 * 2 ** j
        cs_bias[6 + j] = 0.5 * PI
    cp[0:64, 256:296] = v0
    cp[0:64, 296:416] = np.tile(cs_mult.reshape(1, 120), (64, 1))
    cp[0:64, 416:536] = np.tile(cs_bias.reshape(1, 120), (64, 1))
    return cp


def _build_module():
    """Build the (input-independent) Bass module."""
    import concourse.bass as bass
    import concourse.mybir as mybir
    from concourse import bacc
    from concourse.tile import TileContext

    f32 = mybir.dt.float32
    f32r = mybir.dt.float32r
    AF = mybir.ActivationFunctionType
    OP = mybir.AluOpType
    AX = mybir.AxisListType

    nc = bacc.Bacc("TRN2", target_bir_lowering=False, debug=False)

    xin = nc.dram_tensor("xin", [B_CORE, 10], f32, kind="ExternalInput").ap()
    wstack = nc.dram_tensor("wstack", [128, 24 * 128], f32,
                            kind="ExternalInput").ap()
    out_d = nc.dram_tensor("out", [B_CORE, 10], f32, kind="ExternalOutput").ap()

    cpack_c = nc.inline_tensor(_cpack(), name="cpack").ap()
    scrL = nc.dram_tensor("scrL", [2, 64, 32], f32)

    with TileContext(nc) as tc:
        with (
            tc.tile_pool(name="wpool", bufs=1) as wpool,
            tc.tile_pool(name="sb", bufs=2) as sb,
            tc.tile_pool(name="small", bufs=2) as sm,
            tc.tile_pool(name="psA", bufs=1, space="PSUM") as psA,
            tc.tile_pool(name="psB", bufs=1, space="PSUM") as psB,
        ):
            # ---- input DMA + feature-map consts on SP ahead of the big
            # weight DMA; keep the ACT queue clear for table loads
            sx = sm.tile([64, 10], f32, tag="sx")
            nc.sync.dma_start(sx[:], xin)
            ct = wpool.tile([128, 536], f32, tag="cp")
            nc.gpsimd.dma_start(ct[0:64, _CA0:536], cpack_c[0:64, _CA0:536])

            # ---- ACT warm-up: preload the tanh table while sx is in flight
            wu = sm.tile([1, 8], f32, tag="wu")
            nc.vector.memset(wu[:], 0.0)
            wuo = sm.tile([1, 8], f32, tag="wuo")
            nc.scalar.activation(wuo[:], wu[:], AF.Tanh)

            # ---- weights (1.5 MB, linear) on SP; matmul consts on Pool
            wt = wpool.tile([128, 24 * 128], f32, tag="w")
            nc.sync.dma_start(wt[:].bitcast(f32r), wstack.bitcast(f32r))
            nc.gpsimd.dma_start(ct[:, 0:_CA0].bitcast(f32r),
                    cpack_c[:, 0:_CA0].bitcast(f32r))

            def W(m):
                return wt[:, 128 * m:128 * m + 128].bitcast(f32r)

            w1_t = ct[:, 0:64]
            wA_t = ct[:, 64:128]
            e4_t = ct[0:4, 128:256]
            v0_v = ct[0:64, 256:296]
            mult_v = ct[0:64, 296:416]
            bias_v = ct[0:64, 416:536]

            # ---- feature map: x = tanh(xin) (sin table load then overlaps
            # the DVE trig-argument prep)
            xt = sm.tile([64, 10], f32, tag="xt")
            nc.scalar.activation(xt[:], sx[:], AF.Tanh)

            # trig table tb: rows 0-5 = sin(c_j x), 6-11 = -sin, 12-17 = cos
            xb12 = (xt[:].unsqueeze(1).broadcast_to((64, 12, 10)))
            ma = sm.tile([64, 120], f32, tag="ma")
            nc.vector.tensor_tensor(
                ma[:].rearrange("p (r q) -> p r q", q=10), xb12, mult_v
                .rearrange("p (r q) -> p r q", q=10), OP.mult)
            nc.vector.tensor_tensor(ma[:], ma[:], bias_v, OP.add)
            # range reduce to [-pi, pi]: k = round(ma/2pi) via magic-number
            MAGIC = 1.5 * 2 ** 23
            kk = sm.tile([64, 120], f32, tag="kk")
            nc.vector.tensor_scalar(kk[:], ma[:], 1.0 / (2.0 * PI), MAGIC,
                                    OP.mult, OP.add)
            nc.vector.tensor_scalar(kk[:], kk[:], MAGIC, None, OP.subtract)
            nc.vector.scalar_tensor_tensor(ma[:], kk[:], -2.0 * PI, ma[:],
                                           OP.mult, OP.add)
            PCLAMP = PI * (1.0 - 1e-6)
            nc.vector.tensor_scalar(ma[:], ma[:], PCLAMP, -PCLAMP,
                                    OP.min, OP.max)
            tb = sm.tile([64, 180], f32, tag="tb180")
            # sin & cos rows: Sin(ma - pi) trick not needed; args in [-pi,pi]
            nc.scalar.activation(tb[:, 0:60], ma[:, 0:60], AF.Sin)
            nc.scalar.activation(tb[:, 120:180], ma[:, 60:120], AF.Sin)
            # ns rows 6-11 = -s (Pool)
            nc.gpsimd.tensor_scalar(tb[:, 60:120], tb[:, 0:60], -1.0, None,
                                    OP.mult)

            tb_v = tb[:].rearrange("p (r q) -> p r q", q=10)  # [64, 18, 10]
            v_cur = None
            for k in range(10):
                is_rz = (k % 2 == 0)
                lv = k // 2 + 1 if is_rz else k // 2
                cplane = (tb_v[:, 12 + lv, :].unsqueeze(2)
                          .broadcast_to((64, 10, 4))
                          .rearrange("p q (a b) -> p q a b", a=2))
                t1 = sm.tile([64, 40], f32, tag="t1")
                t2 = sm.tile([64, 40], f32, tag="t2")
                t1v = t1[:].rearrange("p (q a b) -> p q a b", a=2, b=2)
                t2v = t2[:].rearrange("p (q a b) -> p q a b", a=2, b=2)
                vsrc = v0_v if v_cur is None else v_cur[:]
                vv = vsrc.rearrange("p (q a b) -> p q a b", a=2, b=2)
                nc.vector.tensor_tensor(t1v, vv, cplane, OP.mult)
                if is_rz:
                    vpart = vv[:, :, :, ::-1]
                    s_alpha = (tb_v[:, lv:lv + 7:6, :].transpose([0, 2, 1])
                               .unsqueeze(2))       # [64, 10, 1, 2] (s, ns)
                    s_beta = (tb_v[:, lv + 6:lv - 1:-6, :]
                              .transpose([0, 2, 1]).unsqueeze(2))
                    nc.gpsimd.tensor_tensor(t2v[:, :, 0:1, :],
                                            vpart[:, :, 0:1, :], s_alpha,
                                            OP.mult)
                    nc.gpsimd.tensor_tensor(t2v[:, :, 1:2, :],
                                            vpart[:, :, 1:2, :], s_beta,
                                            OP.mult)
                else:
                    vpart = vv[:, :, ::-1, ::-1]
                    sview = (tb_v[:, lv:lv + 7:6, :].transpose([0, 2, 1])
                             .unsqueeze(2).broadcast_to((64, 10, 2, 2)))
                    nc.gpsimd.tensor_tensor(t2v, vpart, sview, OP.mult)
                v_nxt = sm.tile([64, 40], f32, tag="vb" if k % 2 == 0 else "va")
                nc.vector.tensor_tensor(v_nxt[:], t1[:], t2[:], OP.add)
                v_cur = v_nxt

            # ---- H/L doubling: G tiles [64, 64], H cols 0:32, L cols 32:64
            g_r = sm.tile([64, 64], f32, tag="gra")
            g_i = sm.tile([64, 64], f32, tag="gia")
            vvq = v_cur[:].rearrange("p (q t c) -> p q t c", t=2, c=2)
            g_r0 = g_r[:].rearrange("p (s x) -> p s x", s=2)[:, :, 0:2]
            g_i0 = g_i[:].rearrange("p (s x) -> p s x", s=2)[:, :, 0:2]
            nc.vector.tensor_copy(g_r0, vvq[:, 4:6, :, 0])
            nc.gpsimd.tensor_copy(g_i0, vvq[:, 4:6, :, 1])
            for j in range(1, 5):
                w = 2 ** j
                qH = H_QUBITS[j]
                ptA = sm.tile([64, 8 * w], f32, tag="ptA")
                ptB = sm.tile([64, 8 * w], f32, tag="ptB")
                gr_b = (g_r[:].rearrange("p (s x) -> p s x", s=2)[:, :, 0:w]
                        .unsqueeze(3).broadcast_to((64, 2, w, 2)))
                gi_b = (g_i[:].rearrange("p (s x) -> p s x", s=2)[:, :, 0:w]
                        .unsqueeze(3).broadcast_to((64, 2, w, 2)))
                vsel = vvq[:, qH:qH + 7:6]          # [64, 2q, 2t, 2c]
                vA = (vsel.transpose([0, 3, 1, 2])  # [64, c(r,i), q, t]
                      .unsqueeze(3).broadcast_to((64, 2, 2, w, 2)))
                vB = (vsel[:, :, :, ::-1].transpose([0, 3, 1, 2])
                      .unsqueeze(3).broadcast_to((64, 2, 2, w, 2)))
                ptA_v = ptA[:].rearrange("p (c s x t) -> p c s x t",
                                         c=2, s=2, t=2)
                ptB_v = ptB[:].rearrange("p (c s x t) -> p c s x t",
                                         c=2, s=2, t=2)
                for c in range(2):
                    nc.vector.tensor_tensor(ptA_v[:, c], gr_b, vA[:, c],
                                            OP.mult)
                    nc.gpsimd.tensor_tensor(ptB_v[:, c], gi_b, vB[:, c],
                                            OP.mult)
                g2_r = sm.tile([64, 64], f32, tag="grb" if j % 2 else "gra")
                g2_i = sm.tile([64, 64], f32, tag="gib" if j % 2 else "gia")
                g2r_v = g2_r[:].rearrange("p (s h t) -> p s h t",
                                          s=2, t=2)[:, :, 0:w, :]
                g2i_v = g2_i[:].rearrange("p (s h t) -> p s h t",
                                          s=2, t=2)[:, :, 0:w, :]
                nc.vector.tensor_tensor(g2r_v, ptA_v[:, 0], ptB_v[:, 0],
                                        OP.subtract)
                nc.gpsimd.tensor_tensor(g2i_v, ptA_v[:, 1], ptB_v[:, 1],
                                        OP.add)
                g_r, g_i = g2_r, g2_i

            # ---- repack.  L half: bounce gL through DRAM, read back as
            # lpp [4=(b5,b4), 512=(u,l)], then E4 replication matmul.
            for comp, g_c, eng in ((0, g_r, nc.sync), (1, g_i, nc.scalar)):
                eng.dma_start(scrL.ap()[comp], g_c[:, 32:64])
            # ---- H half: gt2 [32, 128] (re cols 0:64, im 64:128) via 4
            # StreamTransposes, then 8 dual-comp [16, 32] DMAs -> hp2
            gt2 = sm.tile([32, 128], f32, tag="gt2")
            for comp, g_c in ((0, g_r), (1, g_i)):
                for a in range(2):
                    c0 = 64 * comp + 32 * a
                    nc.vector.transpose(gt2[0:32, c0:c0 + 32],
                                        g_c[32 * a:32 * a + 32, 0:32])
            # hp2 [128 = (b5, q4, b4, h4), 32 = (comp, u4)]
            hp2 = sm.tile([128, 32], f32, tag="hp2")
            _hp_engs = [nc.sync, nc.scalar, nc.gpsimd]
            k = 0
            for b5 in range(2):
                for q4 in range(2):
                    for b4 in range(2):
                        p0 = 64 * b5 + 32 * q4 + 16 * b4
                        c0 = 32 * b5 + 16 * b4
                        dst = (hp2[p0:p0 + 16, :]
                               .rearrange("h (c u) -> h c u", c=2))
                        src = (gt2[16 * q4:16 * q4 + 16, :]
                               .rearrange("h (c x) -> h c x", c=2)
                               [:, :, c0:c0 + 16])
                        _hp_engs[k % 3].dma_start(dst.bitcast(f32r),
                                                  src.bitcast(f32r))
                        k += 1

            lpp_r = sm.tile([4, 512], f32, tag="lppr")
            lpp_i = sm.tile([4, 512], f32, tag="lppi")
            for comp, lpp, eng in ((0, lpp_r, nc.sync),
                                   (1, lpp_i, nc.scalar)):
                src = (scrL.ap()[comp]
                       .rearrange("(c u) l -> c u l", c=4, u=16))
                dst = lpp[:].rearrange("c (u l) -> c u l", u=16, l=32)
                eng.dma_start(dst.bitcast(f32r), src.bitcast(f32r))

            lr_r = psA.tile([128, 512], f32, tag="yr")
            lr_i = psA.tile([128, 512], f32, tag="yi")
            nc.tensor.matmul(lr_r[:], e4_t.bitcast(f32r),
                             lpp_r[:].bitcast(f32r), start=True, stop=True)
            nc.tensor.matmul(lr_i[:], e4_t.bitcast(f32r),
                             lpp_i[:].bitcast(f32r), start=True, stop=True)

            # ---- layer-0 A-side on hp (tiny matmuls): hq = S_A(0) @ hp
            hq_r = psB.tile([128, 16], f32, tag="hqr")
            hq_i = psB.tile([128, 16], f32, tag="hqi")
            hpr_r32 = hp2[:, 0:16].bitcast(f32r)
            hpi_r32 = hp2[:, 16:32].bitcast(f32r)
            nc.tensor.matmul(hq_r[:], W(0), hpr_r32, start=True, stop=False)
            nc.tensor.matmul(hq_r[:], W(1), hpi_r32, start=False, stop=True)
            nc.tensor.matmul(hq_i[:], W(0), hpi_r32, start=True, stop=False)
            nc.tensor.matmul(hq_i[:], W(2), hpr_r32, start=False, stop=True)

            # hq PSUM -> SBUF (engines may read only one PSUM operand)
            hqs_r = sm.tile([128, 16], f32, tag="hqsr")
            hqs_i = sm.tile([128, 16], f32, tag="hqsi")
            nc.vector.tensor_copy(hqs_r[:], hq_r[:])
            nc.scalar.activation(hqs_i[:], hq_i[:], AF.Copy)

            # LRep_i PSUM -> SBUF for the Pool ops (GPSIMD cannot read
            # PSUM); the DVE ops read LRep_r from PSUM directly
            lrs_i = sb.tile([128, 512], f32, tag="lsi")
            nc.scalar.activation(lrs_i[:], lr_i[:], AF.Copy)

            # ---- y1 = hq * LRep (complex), layout A (already A-gated)
            x_r = sb.tile([128, 512], f32, tag="xr")
            x_i = sb.tile([128, 512], f32, tag="xi")
            ta = sb.tile([128, 512], f32, tag="ta")
            tbt = sb.tile([128, 512], f32, tag="tb")
            tct = sb.tile([128, 512], f32, tag="tc")
            tdt = sb.tile([128, 512], f32, tag="td")
            hqr_b = hqs_r[:].unsqueeze(2).broadcast_to((128, 16, 32))
            hqi_b = hqs_i[:].unsqueeze(2).broadcast_to((128, 16, 32))
            lrr_v = lr_r[:].rearrange("p (u l) -> p u l", l=32)
            lri_v = lrs_i[:].rearrange("p (u l) -> p u l", l=32)
            ta_v = ta[:].rearrange("p (u l) -> p u l", l=32)
            tb_v2 = tbt[:].rearrange("p (u l) -> p u l", l=32)
            tc_v = tct[:].rearrange("p (u l) -> p u l", l=32)
            td_v = tdt[:].rearrange("p (u l) -> p u l", l=32)
            xr_v = x_r[:].rearrange("p (u l) -> p u l", l=32)
            xi_v = x_i[:].rearrange("p (u l) -> p u l", l=32)
            nc.vector.tensor_tensor(ta_v, hqr_b, lrr_v, OP.mult)
            nc.gpsimd.tensor_tensor(tb_v2, hqi_b, lri_v, OP.mult)
            nc.vector.tensor_tensor(xr_v.bitcast(f32r), ta_v, tb_v2,
                                    OP.subtract)
            nc.gpsimd.tensor_tensor(tc_v, hqr_b, lri_v, OP.mult)
            nc.vector.tensor_tensor(td_v, hqi_b, lrr_v, OP.mult)
            nc.gpsimd.tensor_tensor(xi_v.bitcast(f32r), tc_v, td_v, OP.add)

            # ---- layers: y (above or A-MMs) -> transpose -> L-MMs -> ...
            # The i-component transpose/copy and the MMs that consume it are
            # split into column halves so the second matmul of each pair can
            # start after only half of b_i is ready.
            def half_flip(src_r, src_i, tag0, tag1, w_a, w_b, w_c, pool,
                          ptag_r, ptag_i):
                """transpose+round src -> (f32r tiles), then 4 accumulating
                MMs into fresh PSUM pair from `pool`: o_r = wa@r + wb@i,
                o_i = wa@i + wc@r.  Returns (o_r, o_i) PSUM tiles."""
                t_r = sb.tile([128, 512], f32, tag="b0r")
                t_i = sb.tile([128, 512], f32, tag="b0i")
                nc.vector.transpose(t_r[:], src_r)
                nc.vector.transpose(t_i[:, 0:256], src_i[:, 0:256])
                nc.vector.transpose(t_i[:, 256:512], src_i[:, 256:512])
                c_r = sb.tile([128, 512], f32, tag=tag0)
                c_i = sb.tile([128, 512], f32, tag=tag1)
                nc.scalar.activation(c_r[:].bitcast(f32r), t_r[:], AF.Copy)
                nc.gpsimd.tensor_copy(c_i[:, 0:256].bitcast(f32r),
                                      t_i[:, 0:256])
                nc.gpsimd.tensor_copy(c_i[:, 256:512].bitcast(f32r),
                                      t_i[:, 256:512])
                o_r = pool.tile([128, 512], f32, tag=ptag_r)
                o_i = pool.tile([128, 512], f32, tag=ptag_i)
                r32 = c_r[:].bitcast(f32r)
                i32 = c_i[:].bitcast(f32r)
                nc.tensor.matmul(o_r[:], w_a, r32, start=True, stop=False)
                nc.tensor.matmul(o_i[:], w_c, r32, start=True, stop=False)
                nc.tensor.matmul(o_r[:], w_b, i32, start=False, stop=True)
                nc.tensor.matmul(o_i[:], w_a, i32, start=False, stop=True)
                return o_r, o_i

            for layer in range(N_LAYERS):
                base = 6 * layer
                if layer == 0:
                    src_r, src_i = x_r[:], x_i[:]
                else:
                    src_r, src_i = zr_ps[:], zi_ps[:]
                zr_ps, zi_ps = half_flip(
                    src_r, src_i, "br", "bi",
                    W(base + 3), W(base + 4), W(base + 5), psB, "zr", "zi")
                if layer < N_LAYERS - 1:
                    nb = 6 * (layer + 1)
                    zr_ps, zi_ps = half_flip(
                        zr_ps[:], zi_ps[:], "xr", "xi",
                        W(nb + 0), W(nb + 1), W(nb + 2), psA, "yr", "yi")

            # ---- measurement.  B path (L qubits): squares on ACT straight
            # from PSUM; A path (H qubits): transpose z first on DVE, square
            # on Pool.  The |z|^2 adds fold into PE accumulation.
            a_r = sb.tile([128, 512], f32, tag="b0r")
            a_i = sb.tile([128, 512], f32, tag="b0i")
            nc.vector.transpose(a_r[:], zr_ps[:])
            nc.vector.transpose(a_i[:], zi_ps[:])
            pB_r = sb.tile([128, 512], f32, tag="pbr")
            pB_i = sb.tile([128, 512], f32, tag="pbi")
            nc.scalar.square(pB_r[:].bitcast(f32r), zr_ps[:])
            nc.scalar.square(pB_i[:].bitcast(f32r), zi_ps[:])
            pA_r = sb.tile([128, 512], f32, tag="par")
            pA_i = sb.tile([128, 512], f32, tag="pai")
            nc.gpsimd.tensor_tensor(pA_r[:].bitcast(f32r), a_r[:], a_r[:],
                                    OP.mult)
            nc.gpsimd.tensor_tensor(pA_i[:].bitcast(f32r), a_i[:], a_i[:],
                                    OP.mult)

            # stage 1a (L qubits, layout B): o1 = W1b.T @ (pB_r + pB_i)
            # [64, 512], rows 32 b5 + j (j=0..4 -> q5..q9)
            o1 = psA.tile([64, 512], f32, tag="o1")
            nc.tensor.matmul(o1[:], w1_t.bitcast(f32r),
                             pB_r[:].bitcast(f32r), start=True, stop=False)
            nc.tensor.matmul(o1[:], w1_t.bitcast(f32r),
                             pB_i[:].bitcast(f32r), start=False, stop=True)
            # stage 1b (H qubits, layout A): o2 = W_A.T @ (pA_r + pA_i)
            # [64, 512], rows 32 b5 + 16 b4 + q (q=0..4)
            o2 = psA.tile([64, 512], f32, tag="o2")
            nc.tensor.matmul(o2[:], wA_t.bitcast(f32r),
                             pA_r[:].bitcast(f32r), start=True, stop=False)
            nc.tensor.matmul(o2[:], wA_t.bitcast(f32r),
                             pA_i[:].bitcast(f32r), start=False, stop=True)

            # plain (h4) sums -> TB [64 = (b5, q'), 32 = 16 b4 + u4]
            TB = sm.tile([64, 32], f32, tag="TB")
            nc.vector.tensor_reduce(
                TB[:].rearrange("p (b4 u) -> p u b4", b4=2),
                o1[:].rearrange("p (u b h) -> p u b h", b=2, h=16),
                AX.X, OP.add)
            # plain (l) sums -> TA [64 = (b5, b4, q), 32] (cols 16:32 zero)
            TA = sm.tile([64, 32], f32, tag="TA")
            nc.gpsimd.memset(TA[:, 16:32], 0.0)
            nc.vector.tensor_reduce(
                TA[:, 0:16],
                o2[:].rearrange("p (u l) -> p u l", l=32),
                AX.X, OP.add)

            # per-block transposes (one op each covers both b5 blocks):
            # finLL[32 b5 + 16 b4 + u4, q'], finHH[32 b5 + u4, 16 b4 + q]
            finLL = sm.tile([64, 32], f32, tag="finL")
            finHH = sm.tile([64, 32], f32, tag="finH")
            nc.vector.transpose(finLL[:], TB[:])
            nc.vector.transpose(finHH[:], TA[:])

            # output DMAs: out[b = 32 b5 + 16 b4 + u4, q] -- one DMA for the
            # L columns, one per b5 for the H columns (DRAM-side rearrange)
            nc.sync.dma_start(out_d[:, 5:10], finLL[0:64, 0:5])
            for b5 in range(2):
                dstH = (out_d[32 * b5:32 * b5 + 32, 0:5]
                        .rearrange("(b4 u) q -> u b4 q", b4=2))
                srcH = (finHH[32 * b5:32 * b5 + 16, :]
                        .rearrange("u (b4 q) -> u b4 q", b4=2)[:, :, 0:5])
                eng = nc.scalar if b5 == 0 else nc.gpsimd
                eng.dma_start(dstH, srcH)

    nc.finalize()
    return nc


def _get_module():
    if "nc" not in _BUILD_CACHE:
        _BUILD_CACHE["nc"] = _build_module()
    return _BUILD_CACHE["nc"]


# ---------------------------------------------------------------- entrypoint
def kernel(inputs, theta):
    inputs = np.asarray(inputs, dtype=np.float32)
    theta = np.asarray(theta, dtype=np.float32)
    assert inputs.shape == (B_TOTAL, N_QUBITS)

    from concourse.bass_utils import run_bass_kernel_spmd

    nc = _get_module()
    wstack = _host_weights(theta)
    in_maps = []
    for c in range(N_CORES):
        shard = np.ascontiguousarray(inputs[B_CORE * c:B_CORE * (c + 1)])
        in_maps.append({"xin": shard, "wstack": wstack})
    res = run_bass_kernel_spmd(nc, in_maps, core_ids=list(range(N_CORES)))
    out = np.concatenate([r["out"] for r in res.results], axis=0)
    return out.astype(np.float32)


# revision 47
# speedup vs baseline: 1.0139x; 1.0139x over previous
"""Trainium2 Bass kernel for nn_EnhancedQuantumLayer (10-qubit, 4-layer
variational circuit, batch 512, Z-expectations output).

Strategy (data parallel over 8 cores, 64 samples/core):
  - Feature map is a product state: per (sample, qubit) 2-vector v computed
    by a 10-step RZ/RX recursion on [64, 40] tiles (DVE/Pool) with sin/cos
    planes from the ACT engine.
  - Statevector [64, 1024] (complex -> separate re/im fp32 planes) held as
    [128, 512] tiles: layout A: partition = (b5, q4, b4, h4), free =
    (u4, l5); layout B (after 32x32 StreamTranspose): partition =
    (b5, q4, l5), free = (u4, b4, h4).  h = q0..q3 (+q4 in partitions),
    l = q5..q9, sample b = (b5, b4, u4).
  - Repack from sample-major g [64, 64] to layout A: L half bounces
    through DRAM (scrL) into lpp [4, 512] + an E4 replication matmul;
    H half via 4 StreamTransposes into gt2 and 8 dual-component
    SBUF->SBUF DMAs into hp2 [128, 32].  Layer-0 A-side gates are applied
    to the tiny hp before the H*L outer product (valid since S_A acts
    only on (q4, h4) and L is constant there).
  - Each layer: 4 fp32r matmuls per side; DVE StreamTranspose (i-comp
    split in column halves) + ACT/Pool f32r rounding copies between
    layouts; CZCNOT entanglers folded into stationaries on host; final
    A-entanglers folded into the measurement signs.
  - Measurement: |z|^2 computed twice (layout B via ACT squares from
    PSUM for L qubits; layout A via DVE transposes + Pool squares for H
    qubits), the re/im adds folded into PE accumulation of the W1b/W_A
    sign matmuls, plain free-dim reductions on DVE, two block
    transposes, 3 output DMAs.

Host precompute is theta-only (24 128x128 stationaries) -- O(1) in batch.
"""

import numpy as np

N_QUBITS = 10
N_LAYERS = 4
FREQS = (1.0, 2.0, 4.0, 8.0, 16.0)
PI = float(np.pi)
B_TOTAL = 512
B_CORE = 64
N_CORES = 8

H_QUBITS = [4, 0, 1, 2, 3]   # kron order (MSB first) for h index
L_QUBITS = [5, 6, 7, 8, 9]

CZCNOT = np.array([[1, 0, 0, 0],
                   [0, 1, 0, 0],
                   [0, 0, 0, -1],
                   [0, 0, 1, 0]], dtype=np.complex128)


# ---------------------------------------------------------------- host math
def _rz(phi):
    return np.array([[np.exp(-0.5j * phi), 0], [0, np.exp(0.5j * phi)]],
                    dtype=np.complex128)


def _rx(th):
    c, s = np.cos(th / 2), np.sin(th / 2)
    return np.array([[c, -1j * s], [-1j * s, c]], dtype=np.complex128)


def _ry(th):
    c, s = np.cos(th / 2), np.sin(th / 2)
    return np.array([[c, -s], [s, c]], dtype=np.complex128)


def _kron_list(ms):
    out = ms[0]
    for m in ms[1:]:
        out = np.kron(out, m)
    return out


def _embed_2q(space_qubits, qa, qb, M4):
    n = len(space_qubits)
    dim = 2 ** n
    pa, pb = space_qubits.index(qa), space_qubits.index(qb)
    out = np.zeros((dim, dim), dtype=np.complex128)
    for idx in range(dim):
        bits = [(idx >> (n - 1 - i)) & 1 for i in range(n)]
        col4 = 2 * bits[pa] + bits[pb]
        for row4 in range(4):
            val = M4[row4, col4]
            if val != 0:
                nb = bits.copy()
                nb[pa], nb[pb] = row4 >> 1, row4 & 1
                ridx = sum(bit << (n - 1 - i) for i, bit in enumerate(nb))
                out[ridx, idx] += val
    return out


A6 = [4, "b4", 0, 1, 2, 3]
L6 = [4, 5, 6, 7, 8, 9]
_E_evenA6 = _embed_2q(A6, 0, 1, CZCNOT) @ _embed_2q(A6, 2, 3, CZCNOT)
_E_oddA6 = _embed_2q(A6, 3, 4, CZCNOT) @ _embed_2q(A6, 1, 2, CZCNOT)
_PermA6 = _E_oddA6 @ _E_evenA6
_E_evenL6 = _embed_2q(L6, 6, 7, CZCNOT) @ _embed_2q(L6, 8, 9, CZCNOT)
_E_oddL6 = _embed_2q(L6, 7, 8, CZCNOT) @ _embed_2q(L6, 5, 6, CZCNOT)
_CG64 = _embed_2q(L6, 4, 5, CZCNOT)


def _layer_matrices6(theta):
    ang = np.tanh(theta.astype(np.float64)) * PI
    S_A, S_L = [], []
    for layer in range(N_LAYERS):
        U = []
        for q in range(10):
            a0, a1, a2 = ang[layer, q]
            U.append(_rx(a0 * 0.5) @ _rz(a2) @ _ry(a1) @ _rz(a0))
        UA6 = _kron_list([U[4], np.eye(2), U[0], U[1], U[2], U[3]])
        UL6 = _kron_list([np.eye(2), U[5], U[6], U[7], U[8], U[9]])
        S_A.append(UA6 if layer == 0 else UA6 @ _PermA6)
        S_L.append(_E_oddL6 @ _E_evenL6 @ _CG64 @ UL6)
    return S_A, S_L


def _host_weights(theta):
    """wstack [128, 24*128] fp32, pre-transposed so the device DMA is
    linear: wstack[p, 128*m + j] = mats[m][p, j], where mats[m] are per
    layer [Ar, Ain, Ai, Lr, Lin, Li], each kron(I2, S6).{comp}.T"""
    S_A, S_L = _layer_matrices6(theta)
    I2 = np.eye(2)
    mats = []
    for layer in range(N_LAYERS):
        for S in [S_A[layer], S_L[layer]]:
            full = np.kron(I2, S)
            mats.append(full.real.T)
            mats.append((-full.imag).T)
            mats.append(full.imag.T)
    stack = np.stack(mats).astype(np.float32)          # [24, 128, 128]
    return np.ascontiguousarray(
        stack.transpose(1, 0, 2).reshape(128, 24 * 128))


def _w1b():
    """W1b [128, 64]: stage-1 (layout B, partitions p = 64 b5 + 32 q4 + l).
    Col 32 b5 + j (j=0..4): sign of l bit j (qubits q5..q9); rest zero."""
    W1 = np.zeros((128, 64), dtype=np.float32)
    for b5 in range(2):
        for q4 in range(2):
            for l in range(32):
                p = 64 * b5 + 32 * q4 + l
                for j in range(5):
                    W1[p, 32 * b5 + j] = 1.0 - 2.0 * ((l >> (4 - j)) & 1)
    return W1


def _wA():
    """W_A [128, 64]: H-qubit measurement in layout A (partitions
    P = 64 b5 + 32 q4 + 16 b4 + h4).  Col 32 b5' + 16 b4' + q (q=0..4):
    [b5==b5'][b4==b4'] * chi_q(q4, h4) with the final A-entanglers folded:
    chi = (s0, s1 s0, s2 s1 s0, s3 s2, s3 s2 (1-2 q4))."""
    W = np.zeros((128, 64), dtype=np.float32)
    for b5 in range(2):
        for q4 in range(2):
            for h4 in range(16):
                s = [1.0 - 2.0 * ((h4 >> (3 - i)) & 1) for i in range(4)]
                chi = [s[0], s[1] * s[0], s[2] * s[1] * s[0], s[3] * s[2],
                       s[3] * s[2] * (1.0 - 2.0 * q4)]
                for b4 in range(2):
                    P = 64 * b5 + 32 * q4 + 16 * b4 + h4
                    for q in range(5):
                        W[P, 32 * b5 + 16 * b4 + q] = chi[q]
    return W


# ------------------------------------------------------------- bass builder
_BUILD_CACHE = {}

# cpack layout (one [128, 536] constant tensor, two DMAs):
#   part B (rows 0..127, cols 0..255): w1b [128,64] | wA [128,64] | e4 [4,128]
#   part A (rows 0..63, cols 256..536): v0 [64,40] | mult [64,120] |
#                                       bias [64,120]
_CA0 = 256          # start of feature-map consts (part A)


def _cpack():
    cp = np.zeros((128, 536), dtype=np.float32)
    # part B
    cp[:, 0:64] = _w1b()
    cp[:, 64:128] = _wA()
    e4 = np.zeros((4, 128), dtype=np.float32)
    for b5 in range(2):
        for b4 in range(2):
            for q4 in range(2):
                p0 = 64 * b5 + 32 * q4 + 16 * b4
                e4[2 * b5 + b4, p0:p0 + 16] = 1.0
    cp[0:4, 128:256] = e4
    # part A
    v0 = np.zeros((64, 40), dtype=np.float32)
    v0[:, 0::4] = 1.0  # alpha_re = 1
    cs_mult = np.zeros((12, 10), dtype=np.float32)
    cs_bias = np.zeros((12, 10), dtype=np.float32)
    for j in range(6):
        cs_mult[j] = 0.25 * 2 ** j
        cs_mult[6 + j] = 0.25 * 2 ** j
        cs_bias[6 + j] = 0.5 * PI
    cp[0:64, 256:296] = v0
    cp[0:64, 296:416] = np.tile(cs_mult.reshape(1, 120), (64, 1))
    cp[0:64, 416:536] = np.tile(cs_bias.reshape(1, 120), (64, 1))
    return cp


def _build_module():
    """Build the (input-independent) Bass module."""
    import concourse.bass as bass
    import concourse.mybir as mybir
    from concourse import bacc
    from concourse.tile import TileContext

    f32 = mybir.dt.float32
    f32r = mybir.dt.float32r
    AF = mybir.ActivationFunctionType
    OP = mybir.AluOpType
    AX = mybir.AxisListType

    nc = bacc.Bacc("TRN2", target_bir_lowering=False, debug=False)

    xin = nc.dram_tensor("xin", [B_CORE, 10], f32, kind="ExternalInput").ap()
    wstack = nc.dram_tensor("wstack", [128, 24 * 128], f32,
                            kind="ExternalInput").ap()
    out_d = nc.dram_tensor("out", [B_CORE, 10], f32, kind="ExternalOutput").ap()

    cpack_c = nc.inline_tensor(_cpack(), name="cpack").ap()
    scrL = nc.dram_tensor("scrL", [2, 64, 32], f32)

    with TileContext(nc) as tc:
        with (
            tc.tile_pool(name="wpool", bufs=1) as wpool,
            tc.tile_pool(name="sb", bufs=2) as sb,
            tc.tile_pool(name="small", bufs=2) as sm,
            tc.tile_pool(name="psA", bufs=1, space="PSUM") as psA,
            tc.tile_pool(name="psB", bufs=1, space="PSUM") as psB,
        ):
            # ---- input DMA + feature-map consts on SP ahead of the big
            # weight DMA; keep the ACT queue clear for table loads
            sx = sm.tile([64, 10], f32, tag="sx")
            nc.sync.dma_start(sx[:], xin)
            ct = wpool.tile([128, 536], f32, tag="cp")
            nc.gpsimd.dma_start(ct[0:64, _CA0:536], cpack_c[0:64, _CA0:536])

            # ---- ACT warm-up: preload the tanh table while sx is in flight
            wu = sm.tile([1, 8], f32, tag="wu")
            nc.vector.memset(wu[:], 0.0)
            wuo = sm.tile([1, 8], f32, tag="wuo")
            nc.scalar.activation(wuo[:], wu[:], AF.Tanh)

            # ---- weights (1.5 MB, linear) on SP; matmul consts on Pool
            wt = wpool.tile([128, 24 * 128], f32, tag="w")
            nc.sync.dma_start(wt[:].bitcast(f32r), wstack.bitcast(f32r))
            nc.gpsimd.dma_start(ct[:, 0:_CA0].bitcast(f32r),
                    cpack_c[:, 0:_CA0].bitcast(f32r))

            def W(m):
                return wt[:, 128 * m:128 * m + 128].bitcast(f32r)

            w1_t = ct[:, 0:64]
            wA_t = ct[:, 64:128]
            e4_t = ct[0:4, 128:256]
            v0_v = ct[0:64, 256:296]
            mult_v = ct[0:64, 296:416]
            bias_v = ct[0:64, 416:536]

            # ---- feature map: x = tanh(xin) (sin table load then overlaps
            # the DVE trig-argument prep)
            xt = sm.tile([64, 10], f32, tag="xt")
            nc.scalar.activation(xt[:], sx[:], AF.Tanh)

            # trig table tb: rows 0-5 = sin(c_j x), 6-11 = -sin, 12-17 = cos
            xb12 = (xt[:].unsqueeze(1).broadcast_to((64, 12, 10)))
            ma = sm.tile([64, 120], f32, tag="ma")
            nc.vector.tensor_tensor(
                ma[:].rearrange("p (r q) -> p r q", q=10), xb12, mult_v
                .rearrange("p (r q) -> p r q", q=10), OP.mult)
            nc.vector.tensor_tensor(ma[:], ma[:], bias_v, OP.add)
            # range reduce to [-pi, pi]: k = round(ma/2pi) via magic-number
            MAGIC = 1.5 * 2 ** 23
            kk = sm.tile([64, 120], f32, tag="kk")
            nc.vector.tensor_scalar(kk[:], ma[:], 1.0 / (2.0 * PI), MAGIC,
                                    OP.mult, OP.add)
            nc.vector.tensor_scalar(kk[:], kk[:], MAGIC, None, OP.subtract)
            nc.vector.scalar_tensor_tensor(ma[:], kk[:], -2.0 * PI, ma[:],
                                           OP.mult, OP.add)
            PCLAMP = PI * (1.0 - 1e-6)
            nc.vector.tensor_scalar(ma[:], ma[:], PCLAMP, -PCLAMP,
                                    OP.min, OP.max)
            tb = sm.tile([64, 180], f32, tag="tb180")
            # sin & cos rows: Sin(ma - pi) trick not needed; args in [-pi,pi]
            nc.scalar.activation(tb[:, 0:60], ma[:, 0:60], AF.Sin)
            nc.scalar.activation(tb[:, 120:180], ma[:, 60:120], AF.Sin)
            # ns rows 6-11 = -s (Pool)
            nc.gpsimd.tensor_scalar(tb[:, 60:120], tb[:, 0:60], -1.0, None,
                                    OP.mult)

            tb_v = tb[:].rearrange("p (r q) -> p r q", q=10)  # [64, 18, 10]
            pewarm = psA.tile([64, 512], f32, tag="o1")
            v_cur = None
            for k in range(10):
                is_rz = (k % 2 == 0)
                lv = k // 2 + 1 if is_rz else k // 2
                cplane = (tb_v[:, 12 + lv, :].unsqueeze(2)
                          .broadcast_to((64, 10, 4))
                          .rearrange("p q (a b) -> p q a b", a=2))
                t1 = sm.tile([64, 40], f32, tag="t1")
                t2 = sm.tile([64, 40], f32, tag="t2")
                t1v = t1[:].rearrange("p (q a b) -> p q a b", a=2, b=2)
                t2v = t2[:].rearrange("p (q a b) -> p q a b", a=2, b=2)
                vsrc = v0_v if v_cur is None else v_cur[:]
                vv = vsrc.rearrange("p (q a b) -> p q a b", a=2, b=2)
                nc.vector.tensor_tensor(t1v, vv, cplane, OP.mult)
                if is_rz:
                    vpart = vv[:, :, :, ::-1]
                    s_alpha = (tb_v[:, lv:lv + 7:6, :].transpose([0, 2, 1])
                               .unsqueeze(2))       # [64, 10, 1, 2] (s, ns)
                    s_beta = (tb_v[:, lv + 6:lv - 1:-6, :]
                              .transpose([0, 2, 1]).unsqueeze(2))
                    nc.gpsimd.tensor_tensor(t2v[:, :, 0:1, :],
                                            vpart[:, :, 0:1, :], s_alpha,
                                            OP.mult)
                    nc.gpsimd.tensor_tensor(t2v[:, :, 1:2, :],
                                            vpart[:, :, 1:2, :], s_beta,
                                            OP.mult)
                else:
                    vpart = vv[:, :, ::-1, ::-1]
                    sview = (tb_v[:, lv:lv + 7:6, :].transpose([0, 2, 1])
                             .unsqueeze(2).broadcast_to((64, 10, 2, 2)))
                    nc.gpsimd.tensor_tensor(t2v, vpart, sview, OP.mult)
                v_nxt = sm.tile([64, 40], f32, tag="vb" if k % 2 == 0 else "va")
                nc.vector.tensor_tensor(v_nxt[:], t1[:], t2[:], OP.add)
                v_cur = v_nxt
                # keep PE p-state ramping (paced dummy fp32 matmul)
                nc.tensor.matmul(pewarm[0:64, 0:40], ct[0:64, 0:64],
                                 v_cur[:], start=True, stop=True)

            # ---- H/L doubling: G tiles [64, 64], H cols 0:32, L cols 32:64
            g_r = sm.tile([64, 64], f32, tag="gra")
            g_i = sm.tile([64, 64], f32, tag="gia")
            vvq = v_cur[:].rearrange("p (q t c) -> p q t c", t=2, c=2)
            g_r0 = g_r[:].rearrange("p (s x) -> p s x", s=2)[:, :, 0:2]
            g_i0 = g_i[:].rearrange("p (s x) -> p s x", s=2)[:, :, 0:2]
            nc.vector.tensor_copy(g_r0, vvq[:, 4:6, :, 0])
            nc.gpsimd.tensor_copy(g_i0, vvq[:, 4:6, :, 1])
            for j in range(1, 5):
                w = 2 ** j
                qH = H_QUBITS[j]
                ptA = sm.tile([64, 8 * w], f32, tag="ptA")
                ptB = sm.tile([64, 8 * w], f32, tag="ptB")
                gr_b = (g_r[:].rearrange("p (s x) -> p s x", s=2)[:, :, 0:w]
                        .unsqueeze(3).broadcast_to((64, 2, w, 2)))
                gi_b = (g_i[:].rearrange("p (s x) -> p s x", s=2)[:, :, 0:w]
                        .unsqueeze(3).broadcast_to((64, 2, w, 2)))
                vsel = vvq[:, qH:qH + 7:6]          # [64, 2q, 2t, 2c]
                vA = (vsel.transpose([0, 3, 1, 2])  # [64, c(r,i), q, t]
                      .unsqueeze(3).broadcast_to((64, 2, 2, w, 2)))
                vB = (vsel[:, :, :, ::-1].transpose([0, 3, 1, 2])
                      .unsqueeze(3).broadcast_to((64, 2, 2, w, 2)))
                ptA_v = ptA[:].rearrange("p (c s x t) -> p c s x t",
                                         c=2, s=2, t=2)
                ptB_v = ptB[:].rearrange("p (c s x t) -> p c s x t",
                                         c=2, s=2, t=2)
                for c in range(2):
                    nc.vector.tensor_tensor(ptA_v[:, c], gr_b, vA[:, c],
                                            OP.mult)
                    nc.gpsimd.tensor_tensor(ptB_v[:, c], gi_b, vB[:, c],
                                            OP.mult)
                g2_r = sm.tile([64, 64], f32, tag="grb" if j % 2 else "gra")
                g2_i = sm.tile([64, 64], f32, tag="gib" if j % 2 else "gia")
                g2r_v = g2_r[:].rearrange("p (s h t) -> p s h t",
                                          s=2, t=2)[:, :, 0:w, :]
                g2i_v = g2_i[:].rearrange("p (s h t) -> p s h t",
                                          s=2, t=2)[:, :, 0:w, :]
                nc.vector.tensor_tensor(g2r_v, ptA_v[:, 0], ptB_v[:, 0],
                                        OP.subtract)
                nc.gpsimd.tensor_tensor(g2i_v, ptA_v[:, 1], ptB_v[:, 1],
                                        OP.add)
                g_r, g_i = g2_r, g2_i
                nc.tensor.matmul(pewarm[0:64, 0:8 * w], ct[0:64, 0:64],
                                 ptA[:], start=True, stop=True)

            # ---- repack.  L half: bounce gL through DRAM, read back as
            # lpp [4=(b5,b4), 512=(u,l)], then E4 replication matmul.
            for comp, g_c, eng in ((0, g_r, nc.sync), (1, g_i, nc.scalar)):
                eng.dma_start(scrL.ap()[comp], g_c[:, 32:64])
            # ---- H half: gt2 [32, 128] (re cols 0:64, im 64:128) via 4
            # StreamTransposes, then 8 dual-comp [16, 32] DMAs -> hp2
            gt2 = sm.tile([32, 128], f32, tag="gt2")
            for comp, g_c in ((0, g_r), (1, g_i)):
                for a in range(2):
                    c0 = 64 * comp + 32 * a
                    nc.vector.transpose(gt2[0:32, c0:c0 + 32],
                                        g_c[32 * a:32 * a + 32, 0:32])
            # hp2 [128 = (b5, q4, b4, h4), 32 = (comp, u4)]
            hp2 = sm.tile([128, 32], f32, tag="hp2")
            _hp_engs = [nc.sync, nc.scalar, nc.gpsimd]
            k = 0
            for b5 in range(2):
                for q4 in range(2):
                    for b4 in range(2):
                        p0 = 64 * b5 + 32 * q4 + 16 * b4
                        c0 = 32 * b5 + 16 * b4
                        dst = (hp2[p0:p0 + 16, :]
                               .rearrange("h (c u) -> h c u", c=2))
                        src = (gt2[16 * q4:16 * q4 + 16, :]
                               .rearrange("h (c x) -> h c x", c=2)
                               [:, :, c0:c0 + 16])
                        _hp_engs[k % 3].dma_start(dst.bitcast(f32r),
                                                  src.bitcast(f32r))
                        k += 1

            lpp_r = sm.tile([4, 512], f32, tag="lppr")
            lpp_i = sm.tile([4, 512], f32, tag="lppi")
            for comp, lpp, eng in ((0, lpp_r, nc.sync),
                                   (1, lpp_i, nc.scalar)):
                src = (scrL.ap()[comp]
                       .rearrange("(c u) l -> c u l", c=4, u=16))
                dst = lpp[:].rearrange("c (u l) -> c u l", u=16, l=32)
                eng.dma_start(dst.bitcast(f32r), src.bitcast(f32r))

            lr_r = psA.tile([128, 512], f32, tag="yr")
            lr_i = psA.tile([128, 512], f32, tag="yi")
            nc.tensor.matmul(lr_r[:], e4_t.bitcast(f32r),
                             lpp_r[:].bitcast(f32r), start=True, stop=True)
            nc.tensor.matmul(lr_i[:], e4_t.bitcast(f32r),
                             lpp_i[:].bitcast(f32r), start=True, stop=True)

            # ---- layer-0 A-side on hp (tiny matmuls): hq = S_A(0) @ hp
            hq_r = psB.tile([128, 16], f32, tag="hqr")
            hq_i = psB.tile([128, 16], f32, tag="hqi")
            hpr_r32 = hp2[:, 0:16].bitcast(f32r)
            hpi_r32 = hp2[:, 16:32].bitcast(f32r)
            nc.tensor.matmul(hq_r[:], W(0), hpr_r32, start=True, stop=False)
            nc.tensor.matmul(hq_r[:], W(1), hpi_r32, start=False, stop=True)
            nc.tensor.matmul(hq_i[:], W(0), hpi_r32, start=True, stop=False)
            nc.tensor.matmul(hq_i[:], W(2), hpr_r32, start=False, stop=True)

            # hq PSUM -> SBUF (engines may read only one PSUM operand)
            hqs_r = sm.tile([128, 16], f32, tag="hqsr")
            hqs_i = sm.tile([128, 16], f32, tag="hqsi")
            nc.vector.tensor_copy(hqs_r[:], hq_r[:])
            nc.scalar.activation(hqs_i[:], hq_i[:], AF.Copy)

            # LRep_i PSUM -> SBUF for the Pool ops (GPSIMD cannot read
            # PSUM); the DVE ops read LRep_r from PSUM directly
            lrs_i = sb.tile([128, 512], f32, tag="lsi")
            nc.scalar.activation(lrs_i[:], lr_i[:], AF.Copy)

            # ---- y1 = hq * LRep (complex), layout A (already A-gated)
            x_r = sb.tile([128, 512], f32, tag="xr")
            x_i = sb.tile([128, 512], f32, tag="xi")
            ta = sb.tile([128, 512], f32, tag="ta")
            tbt = sb.tile([128, 512], f32, tag="tb")
            tct = sb.tile([128, 512], f32, tag="tc")
            tdt = sb.tile([128, 512], f32, tag="td")
            hqr_b = hqs_r[:].unsqueeze(2).broadcast_to((128, 16, 32))
            hqi_b = hqs_i[:].unsqueeze(2).broadcast_to((128, 16, 32))
            lrr_v = lr_r[:].rearrange("p (u l) -> p u l", l=32)
            lri_v = lrs_i[:].rearrange("p (u l) -> p u l", l=32)
            ta_v = ta[:].rearrange("p (u l) -> p u l", l=32)
            tb_v2 = tbt[:].rearrange("p (u l) -> p u l", l=32)
            tc_v = tct[:].rearrange("p (u l) -> p u l", l=32)
            td_v = tdt[:].rearrange("p (u l) -> p u l", l=32)
            xr_v = x_r[:].rearrange("p (u l) -> p u l", l=32)
            xi_v = x_i[:].rearrange("p (u l) -> p u l", l=32)
            nc.vector.tensor_tensor(ta_v, hqr_b, lrr_v, OP.mult)
            nc.gpsimd.tensor_tensor(tb_v2, hqi_b, lri_v, OP.mult)
            nc.vector.tensor_tensor(xr_v.bitcast(f32r), ta_v, tb_v2,
                                    OP.subtract)
            nc.gpsimd.tensor_tensor(tc_v, hqr_b, lri_v, OP.mult)
            nc.vector.tensor_tensor(td_v, hqi_b, lrr_v, OP.mult)
            nc.gpsimd.tensor_tensor(xi_v.bitcast(f32r), tc_v, td_v, OP.add)

            # ---- layers: y (above or A-MMs) -> transpose -> L-MMs -> ...
            # The i-component transpose/copy and the MMs that consume it are
            # split into column halves so the second matmul of each pair can
            # start after only half of b_i is ready.
            def half_flip(src_r, src_i, tag0, tag1, w_a, w_b, w_c, pool,
                          ptag_r, ptag_i):
                """transpose+round src -> (f32r tiles), then 4 accumulating
                MMs into fresh PSUM pair from `pool`: o_r = wa@r + wb@i,
                o_i = wa@i + wc@r.  Returns (o_r, o_i) PSUM tiles."""
                t_r = sb.tile([128, 512], f32, tag="b0r")
                t_i = sb.tile([128, 512], f32, tag="b0i")
                nc.vector.transpose(t_r[:], src_r)
                nc.vector.transpose(t_i[:, 0:256], src_i[:, 0:256])
                nc.vector.transpose(t_i[:, 256:512], src_i[:, 256:512])
                c_r = sb.tile([128, 512], f32, tag=tag0)
                c_i = sb.tile([128, 512], f32, tag=tag1)
                nc.scalar.activation(c_r[:].bitcast(f32r), t_r[:], AF.Copy)
                nc.gpsimd.tensor_copy(c_i[:, 0:256].bitcast(f32r),
                                      t_i[:, 0:256])
                nc.gpsimd.tensor_copy(c_i[:, 256:512].bitcast(f32r),
                                      t_i[:, 256:512])
                o_r = pool.tile([128, 512], f32, tag=ptag_r)
                o_i = pool.tile([128, 512], f32, tag=ptag_i)
                r32 = c_r[:].bitcast(f32r)
                i32 = c_i[:].bitcast(f32r)
                nc.tensor.matmul(o_r[:], w_a, r32, start=True, stop=False)
                nc.tensor.matmul(o_i[:], w_c, r32, start=True, stop=False)
                nc.tensor.matmul(o_r[:], w_b, i32, start=False, stop=True)
                nc.tensor.matmul(o_i[:], w_a, i32, start=False, stop=True)
                return o_r, o_i

            for layer in range(N_LAYERS):
                base = 6 * layer
                if layer == 0:
                    src_r, src_i = x_r[:], x_i[:]
                else:
                    src_r, src_i = zr_ps[:], zi_ps[:]
                zr_ps, zi_ps = half_flip(
                    src_r, src_i, "br", "bi",
                    W(base + 3), W(base + 4), W(base + 5), psB, "zr", "zi")
                if layer < N_LAYERS - 1:
                    nb = 6 * (layer + 1)
                    zr_ps, zi_ps = half_flip(
                        zr_ps[:], zi_ps[:], "xr", "xi",
                        W(nb + 0), W(nb + 1), W(nb + 2), psA, "yr", "yi")

            # ---- measurement.  B path (L qubits): squares on ACT straight
            # from PSUM; A path (H qubits): transpose z first on DVE, square
            # on Pool.  The |z|^2 adds fold into PE accumulation.
            a_r = sb.tile([128, 512], f32, tag="b0r")
            a_i = sb.tile([128, 512], f32, tag="b0i")
            nc.vector.transpose(a_r[:], zr_ps[:])
            nc.vector.transpose(a_i[:], zi_ps[:])
            pB_r = sb.tile([128, 512], f32, tag="pbr")
            pB_i = sb.tile([128, 512], f32, tag="pbi")
            nc.scalar.square(pB_r[:].bitcast(f32r), zr_ps[:])
            nc.scalar.square(pB_i[:].bitcast(f32r), zi_ps[:])
            pA_r = sb.tile([128, 512], f32, tag="par")
            pA_i = sb.tile([128, 512], f32, tag="pai")
            nc.gpsimd.tensor_tensor(pA_r[:].bitcast(f32r), a_r[:], a_r[:],
                                    OP.mult)
            nc.gpsimd.tensor_tensor(pA_i[:].bitcast(f32r), a_i[:], a_i[:],
                                    OP.mult)

            # stage 1a (L qubits, layout B): o1 = W1b.T @ (pB_r + pB_i)
            # [64, 512], rows 32 b5 + j (j=0..4 -> q5..q9)
            o1 = psA.tile([64, 512], f32, tag="o1")
            nc.tensor.matmul(o1[:], w1_t.bitcast(f32r),
                             pB_r[:].bitcast(f32r), start=True, stop=False)
            nc.tensor.matmul(o1[:], w1_t.bitcast(f32r),
                             pB_i[:].bitcast(f32r), start=False, stop=True)
            # stage 1b (H qubits, layout A): o2 = W_A.T @ (pA_r + pA_i)
            # [64, 512], rows 32 b5 + 16 b4 + q (q=0..4)
            o2 = psA.tile([64, 512], f32, tag="o2")
            nc.tensor.matmul(o2[:], wA_t.bitcast(f32r),
                             pA_r[:].bitcast(f32r), start=True, stop=False)
            nc.tensor.matmul(o2[:], wA_t.bitcast(f32r),
                             pA_i[:].bitcast(f32r), start=False, stop=True)

            # plain (h4) sums -> TB [64 = (b5, q'), 32 = 16 b4 + u4]
            TB = sm.tile([64, 32], f32, tag="TB")
            nc.vector.tensor_reduce(
                TB[:].rearrange("p (b4 u) -> p u b4", b4=2),
                o1[:].rearrange("p (u b h) -> p u b h", b=2, h=16),
                AX.X, OP.add)
            # plain (l) sums -> TA [64 = (b5, b4, q), 32] (cols 16:32 zero)
            TA = sm.tile([64, 32], f32, tag="TA")
            nc.gpsimd.memset(TA[:, 16:32], 0.0)
            nc.vector.tensor_reduce(
                TA[:, 0:16],
                o2[:].rearrange("p (u l) -> p u l", l=32),
                AX.X, OP.add)

            # per-block transposes (one op each covers both b5 blocks):
            # finLL[32 b5 + 16 b4 + u4, q'], finHH[32 b5 + u4, 16 b4 + q]
            finLL = sm.tile([64, 32], f32, tag="finL")
            finHH = sm.tile([64, 32], f32, tag="finH")
            nc.vector.transpose(finLL[:], TB[:])
            nc.vector.transpose(finHH[:], TA[:])

            # output DMAs: out[b = 32 b5 + 16 b4 + u4, q] -- one DMA for the
            # L columns, one per b5 for the H columns (DRAM-side rearrange)
            nc.sync.dma_start(out_d[:, 5:10], finLL[0:64, 0:5])
            for b5 in range(2):
                dstH = (out_d[32 * b5:32 * b5 + 32, 0:5]
                        .rearrange("(b4 u) q -> u b4 q", b4=2))
                srcH = (finHH[32 * b5:32 * b5 + 16, :]
                        .rearrange("u (b4 q) -> u b4 q", b4=2)[:, :, 0:5])
                eng = nc.scalar if b5 == 0 else nc.gpsimd
                eng.dma_start(dstH, srcH)

    nc.finalize()
    return nc


def _get_module():
    if "nc" not in _BUILD_CACHE:
        _BUILD_CACHE["nc"] = _build_module()
    return _BUILD_CACHE["nc"]


# ---------------------------------------------------------------- entrypoint
def kernel(inputs, theta):
    inputs = np.asarray(inputs, dtype=np.float32)
    theta = np.asarray(theta, dtype=np.float32)
    assert inputs.shape == (B_TOTAL, N_QUBITS)

    from concourse.bass_utils import run_bass_kernel_spmd

    nc = _get_module()
    wstack = _host_weights(theta)
    in_maps = []
    for c in range(N_CORES):
        shard = np.ascontiguousarray(inputs[B_CORE * c:B_CORE * (c + 1)])
        in_maps.append({"xin": shard, "wstack": wstack})
    res = run_bass_kernel_spmd(nc, in_maps, core_ids=list(range(N_CORES)))
    out = np.concatenate([r["out"] for r in res.results], axis=0)
    return out.astype(np.float32)
